# revision 22
# baseline (speedup 1.0000x reference)
"""Trainium2 Bass kernel for nn_CrossModalAttention (B=4, Sq=Sk=2048, D=512, H=8).

Self-contained 8-core SPMD program: core c handles batch c//2, query-half c%2
(SQ=1024 of 2048 queries). Cores fully independent, no collectives.

Schedule (v2): the ACT engine's exp over the score matrix (H*SQ*SK elements,
128 instrs of [128,1024]) is the hard floor (~133us busy). Everything else is
arranged so ACT runs saturated from ~5us on:
  - K/V/Q projections are split into fine-grained PSUM chains (K per
    (dc,512-key window), V per (128-key chunk, 4-head group), Q per (dc,
    512-query half)) emitted in deadline order BEFORE the attention loop;
    the Tile list-scheduler trickles them into PE slack between St/PV work.
  - PSUM: St double-buffered (4 banks) + per-head context accumulator
    (1 bank, 8 qs x 64 dk) + row-sum L accumulator (1 bank, via [128,1]
    matmuls against a ones column) + 2 projection-chain banks = 8.
  - C stored bf16 so the PE transposes run 1 cycle/row; transposes for head
    pair p woven in after head 2p+1 completes (reusing projection banks).
  - DVE keeps PSUM-reading work (bias adds, normalize, LN stats); the idle
    GPSIMD engine precomputes resid+bo and does half the LN finals.
  - LayerNorm final scale fused to 2 scalar_tensor_tensor ops per row-tile.
  - Input DMAs issued in parallel across SP/ACT/DVE/POOL queues.
"""
import sys
sys.path.insert(0, "/opt/trn_rl_repo")
import numpy as np

from contextlib import ExitStack

import concourse.bass as bass
import concourse.mybir as mybir
import concourse.tile as tile
from concourse import bacc
from concourse.masks import make_identity

FP32 = mybir.dt.float32
BF16 = mybir.dt.bfloat16
P = 128


def build(SQ=1024, SK=2048, D=512, H=8, num_devices=8, dbg=False):
    DK = D // H                   # 64
    DC = D // P                   # 4
    KC = SK // P                  # 16 key chunks
    NQT = SQ // P                 # 8 query subtiles
    QF = 512
    NQF = SQ // QF                # 2
    NW = SK // 512                # 4 key windows
    HG = 2                        # head groups (4 heads each) for V chains
    HPG = H // HG                 # 4
    Alu = mybir.AluOpType
    Act = mybir.ActivationFunctionType

    nc = bacc.Bacc("TRN2", target_bir_lowering=False, debug=False,
                   num_devices=num_devices)

    def din(name, shape, dt=FP32):
        return nc.dram_tensor(name, list(shape), dt, kind="ExternalInput").ap()

    qT = din("qT", (D, SQ), BF16)
    kT = din("kT", (D, SK), BF16)
    vT = din("vT", (D, SK), BF16)
    resid = din("resid", (SQ, D))
    w_dram = {n: din(n, (D, D), BF16) for n in ("wq", "wk", "wv", "wo")}
    # packed consts: [bq 0:4][bk 4:8][bv 8:520][bo 520:1032][g 1032:1544][b 1544:2056]
    cpack = din("cpack", (P, 2 * DC + 4 * D))
    out = nc.dram_tensor("out", [SQ, D], FP32, kind="ExternalOutput").ap()

    with tile.TileContext(nc) as tc, ExitStack() as ctx:
        consts = ctx.enter_context(tc.tile_pool(name="consts", bufs=1))
        acts = ctx.enter_context(tc.tile_pool(name="acts", bufs=1))
        pj_ps = ctx.enter_context(tc.tile_pool(name="pj_ps", bufs=2, space="PSUM"))
        st_ps = ctx.enter_context(tc.tile_pool(name="st_ps", bufs=2, space="PSUM"))
        c_ps = ctx.enter_context(tc.tile_pool(name="c_ps", bufs=1, space="PSUM"))
        pt_pool = ctx.enter_context(tc.tile_pool(name="pt", bufs=4))
        lr_pool = ctx.enter_context(tc.tile_pool(name="lr", bufs=2))
        ep = ctx.enter_context(tc.tile_pool(name="ep", bufs=3))
        ot_pool = ctx.enter_context(tc.tile_pool(name="ot", bufs=2))

        # ---------------- input DMAs, spread across engine queues ----------------
        qT_sb = acts.tile([P, DC, SQ], BF16, tag="qTin")
        qTr = qT.rearrange("(c p) q -> p c q", p=P)
        nc.sync.dma_start(qT_sb[:, :, 0:QF], qTr[:, :, 0:QF])
        nc.scalar.dma_start(qT_sb[:, :, QF:SQ], qTr[:, :, QF:SQ])

        wsb = {}
        for n in ("wq", "wk", "wv", "wo"):
            wsb[n] = acts.tile([P, DC, D], BF16, tag=n, name=n)
        nc.sync.dma_start(wsb["wq"][:], w_dram["wq"].rearrange("(c p) o -> p c o", p=P))

        kin = acts.tile([P, DC, SK], BF16, tag="kin")
        kTr = kT.rearrange("(c p) s -> p c s", p=P)
        nc.scalar.dma_start(kin[:, :, 0:512], kTr[:, :, 0:512])
        nc.scalar.dma_start(wsb["wk"][:], w_dram["wk"].rearrange("(c p) o -> p c o", p=P))

        vin = acts.tile([P, DC, SK], BF16, tag="vin")
        vTr = vT.rearrange("(c p) s -> p c s", p=P)
        nc.gpsimd.dma_start(vin[:, :, 0:512], vTr[:, :, 0:512])
        nc.sync.dma_start(wsb["wv"][:], w_dram["wv"].rearrange("(c p) o -> p c o", p=P))

        # consts: only bq/bk/bv needed during projections; bo/g/b loaded late
        cp_sb = consts.tile([P, 2 * DC + 4 * D], FP32, tag="cpack")
        nc.scalar.dma_start(cp_sb[:, 0:2 * DC + D], cpack[:, 0:2 * DC + D])

        bq_sb = cp_sb[:, 0:DC]
        bk_sb = cp_sb[:, DC:2 * DC]
        bv_sb = cp_sb[:, 2 * DC:2 * DC + D]
        bo_sb = cp_sb[:, 2 * DC + D:2 * DC + 2 * D]
        g_sb = cp_sb[:, 2 * DC + 2 * D:2 * DC + 3 * D]
        b_sb = cp_sb[:, 2 * DC + 3 * D:2 * DC + 4 * D]

        ident = consts.tile([P, P], BF16, tag="ident")
        make_identity(nc, ident[:])
        ones_bf = consts.tile([P, 1], BF16, tag="ones")
        nc.vector.memset(ones_bf[:], 1.0)
        eps_sb = consts.tile([P, 1], FP32, tag="eps")
        nc.vector.memset(eps_sb[:], 1e-5)

        # ---------------- resident activations ----------------
        Qt = acts.tile([P, DC, SQ], BF16, tag="Qt")
        Kt = acts.tile([P, DC, SK], BF16, tag="Kt")
        V = acts.tile([P, KC, H, DK + 1], BF16, tag="V")
        nc.vector.memset(V[:, :, :, DK], 1.0)
        C = acts.tile([P, NQT, D], BF16, tag="C")
        Ct = acts.tile([P, DC, SQ], BF16, tag="Ct")
        t0_all = acts.tile([P, NQT, D], FP32, tag="t0")
        rs2_all = acts.tile([P, NQT, D], FP32, tag="rs2")
        mv_all = acts.tile([P, NQT, 2], FP32, tag="mv")
        sdev = acts.tile([P, NQT], FP32, tag="sdev")
        rstd_all = acts.tile([P, NQT], FP32, tag="rstd")

        # ---------------- projection chains ----------------
        def q_chain(dc, qf):
            ps = pj_ps.tile([P, D], FP32, tag="pj", name=f"q{dc}_{qf}")[:, 0:QF]
            for i in range(DC):
                nc.tensor.matmul(ps, lhsT=wsb["wq"][:, i, dc * P:(dc + 1) * P],
                                 rhs=qT_sb[:, i, qf * QF:(qf + 1) * QF],
                                 start=(i == 0), stop=(i == DC - 1))
            nc.vector.tensor_scalar_add(Qt[:, dc, qf * QF:(qf + 1) * QF], ps,
                                        bq_sb[:, dc:dc + 1])

        def k_chain(dc, w):
            ps = pj_ps.tile([P, D], FP32, tag="pj", name=f"k{dc}_{w}")[:, 0:512]
            for i in range(DC):
                nc.tensor.matmul(ps, lhsT=wsb["wk"][:, i, dc * P:(dc + 1) * P],
                                 rhs=kin[:, i, w * 512:(w + 1) * 512],
                                 start=(i == 0), stop=(i == DC - 1))
            nc.vector.tensor_scalar_add(Kt[:, dc, w * 512:(w + 1) * 512], ps,
                                        bk_sb[:, dc:dc + 1])

        def v_chain(sc, hg):
            cw = hg * HPG * DK                      # 256-col offset
            ps = pj_ps.tile([P, D], FP32, tag="pj",
                            name=f"v{sc}_{hg}")[:, 0:HPG * DK]
            for i in range(DC):
                nc.tensor.matmul(ps, lhsT=vin[:, i, sc * P:(sc + 1) * P],
                                 rhs=wsb["wv"][:, i, cw:cw + HPG * DK],
                                 start=(i == 0), stop=(i == DC - 1))
            nc.vector.tensor_tensor(
                V[:, sc, hg * HPG:(hg + 1) * HPG, 0:DK],
                ps.rearrange("p (h d) -> p h d", d=DK),
                bv_sb[:, cw:cw + HPG * DK].rearrange("p (h d) -> p h d", d=DK),
                Alu.add)

        # prologue: minimum to start (h0, kc0)
        q_chain(0, 0)
        q_chain(0, 1)
        k_chain(0, 0)
        for sc in range(4):
            v_chain(sc, 0)
        # background, deadline order (scheduler trickles into PE slack);
        # later kin/vin windows DMA'd just ahead of their first consumer
        nc.scalar.dma_start(kin[:, :, 512:1024], kTr[:, :, 512:1024])
        nc.gpsimd.dma_start(vin[:, :, 512:1024], vTr[:, :, 512:1024])
        k_chain(0, 1)
        for sc in range(4, 8):
            v_chain(sc, 0)
        nc.scalar.dma_start(kin[:, :, 1024:1536], kTr[:, :, 1024:1536])
        nc.gpsimd.dma_start(vin[:, :, 1024:1536], vTr[:, :, 1024:1536])
        k_chain(0, 2)
        for sc in range(8, 12):
            v_chain(sc, 0)
        nc.scalar.dma_start(kin[:, :, 1536:2048], kTr[:, :, 1536:2048])
        nc.gpsimd.dma_start(vin[:, :, 1536:2048], vTr[:, :, 1536:2048])
        k_chain(0, 3)
        for sc in range(12, 16):
            v_chain(sc, 0)
        q_chain(1, 0)
        q_chain(1, 1)
        for w in range(NW):
            k_chain(1, w)
        for sc in range(16):
            v_chain(sc, 1)
        q_chain(2, 0)
        q_chain(2, 1)
        for w in range(NW):
            k_chain(2, w)
        q_chain(3, 0)
        q_chain(3, 1)
        for w in range(NW):
            k_chain(3, w)

        # ---------------- attention ----------------
        JB = 4                      # qs per cps bank (65 f32 each)
        NQB = NQT // JB

        def emit_transpose(dc):
            for qs in range(NQT):
                tp = pj_ps.tile([P, P], BF16, tag="pj", name=f"tp{dc}_{qs}")
                nc.tensor.transpose(tp[:], C[:, qs, dc * P:(dc + 1) * P], ident[:])
                nc.vector.tensor_copy(out=Ct[:, dc, qs * P:(qs + 1) * P], in_=tp[:])

        for h in range(H):
            dc_h = h // 2
            off = (h % 2) * DK
            cps = c_ps.tile([P, NQB, 512], FP32, tag="cps", name=f"cps{h}")
            for kc in range(KC):
                st = st_ps.tile([P, SQ], FP32, tag="st")
                for qf in range(NQF):
                    nc.tensor.matmul(
                        st[:, qf * QF:(qf + 1) * QF],
                        lhsT=Kt[off:off + DK, dc_h, kc * P:(kc + 1) * P],
                        rhs=Qt[off:off + DK, dc_h, qf * QF:(qf + 1) * QF],
                        start=True, stop=True)
                pt = pt_pool.tile([P, SQ], BF16, tag="pt")
                nc.scalar.activation(pt[:], st[:], Act.Exp, scale=0.125)
                for qs in range(NQT):
                    jcol = (qs % JB) * (DK + 1)
                    nc.tensor.matmul(cps[:, qs // JB, jcol:jcol + DK + 1],
                                     lhsT=pt[:, qs * P:(qs + 1) * P],
                                     rhs=V[:, kc, h, :],
                                     start=(kc == 0 and qs % JB == 0),
                                     stop=(kc == KC - 1 and qs % JB == JB - 1))
            cview = cps[:, :, 0:JB * (DK + 1)].rearrange(
                "p b (j x) -> p b j x", x=DK + 1)
            lr = lr_pool.tile([P, NQB, JB, 1], FP32, tag="lr")
            nc.vector.reciprocal(lr[:], cview[:, :, :, DK:DK + 1])
            nc.vector.tensor_tensor(
                C[:, :, h * DK:(h + 1) * DK].rearrange(
                    "p (b j) d -> p b j d", j=JB),
                cview[:, :, :, 0:DK],
                lr[:].to_broadcast((P, NQB, JB, DK)),
                Alu.mult)
            if h == 5:
                # prefetch epilogue inputs while attention still runs
                nc.gpsimd.dma_start(cp_sb[:, 2 * DC + D:],
                                    cpack[:, 2 * DC + D:])
                nc.gpsimd.dma_start(wsb["wo"][:],
                                    w_dram["wo"].rearrange("(c p) o -> p c o", p=P))
                for qs in range(NQT):
                    rs = ep.tile([P, D], FP32, tag="rs")
                    nc.sync.dma_start(rs[:], resid[qs * P:(qs + 1) * P, :])
                    nc.gpsimd.tensor_tensor(rs2_all[:, qs, :], rs[:], bo_sb,
                                            Alu.add)
            if h % 2 == 1:
                emit_transpose(h // 2)

        # ---------------- out-proj + layernorm ----------------
        def ln_final(qs):
            y = ep.tile([P, D], FP32, tag="y")
            nc.vector.scalar_tensor_tensor(
                y[:], t0_all[:, qs, :], mv_all[:, qs, 0:1],
                g_sb, Alu.subtract, Alu.mult)
            ot = ot_pool.tile([P, D], FP32, tag="ot")
            nc.vector.scalar_tensor_tensor(
                ot[:], y[:], rstd_all[:, qs:qs + 1],
                b_sb, Alu.mult, Alu.add)
            nc.sync.dma_start(out[qs * P:(qs + 1) * P, :], ot[:])

        HQ = NQT // 2
        for qs in range(NQT):
            ps = st_ps.tile([P, SQ], FP32, tag="st", name=f"ops{qs}")[:, 0:D]
            for dc in range(DC):
                nc.tensor.matmul(ps, lhsT=Ct[:, dc, qs * P:(qs + 1) * P],
                                 rhs=wsb["wo"][:, dc, :],
                                 start=(dc == 0), stop=(dc == DC - 1))
            t0 = t0_all[:, qs, :]
            nc.vector.tensor_tensor(t0, ps, rs2_all[:, qs, :], Alu.add)
            st6 = ep.tile([P, 6], FP32, tag="st6")
            nc.vector.bn_stats(st6[:], t0)
            nc.vector.bn_aggr(mv_all[:, qs, :], st6[:])
            if qs == HQ - 1:
                # finalize first half while PE runs the remaining chains
                nc.scalar.activation(sdev[:, 0:HQ], mv_all[:, 0:HQ, 1],
                                     Act.Sqrt, bias=eps_sb[:])
                nc.vector.reciprocal(rstd_all[:, 0:HQ], sdev[:, 0:HQ])
                for q2 in range(HQ):
                    ln_final(q2)

        nc.scalar.activation(sdev[:, HQ:NQT], mv_all[:, HQ:NQT, 1],
                             Act.Sqrt, bias=eps_sb[:])
        nc.vector.reciprocal(rstd_all[:, HQ:NQT], sdev[:, HQ:NQT])
        for qs in range(HQ, NQT):
            ln_final(qs)

        if dbg:
            for nm, t in (("dbg_C", C), ("dbg_Ct", Ct)):
                dt_ = nc.dram_tensor(nm, list(t[:].shape), BF16,
                                     kind="ExternalOutput").ap()
                nc.sync.dma_start(dt_, t[:])
            for nm, t in (("dbg_t0", t0_all), ("dbg_rs2", rs2_all),
                          ("dbg_mv", mv_all), ("dbg_rstd", rstd_all)):
                dt_ = nc.dram_tensor(nm, list(t[:].shape), FP32,
                                     kind="ExternalOutput").ap()
                nc.sync.dma_start(dt_, t[:])

    nc.compile()
    return nc


def make_in_map(query_slice, key_b, value_b, wq, bq, wk, bk, wv, bv, wo, bo,
                ln_g, ln_b):
    """Host-side shard prep for one core. query_slice [SQ, D]; key_b/value_b [SK, D]."""
    import ml_dtypes
    D = wq.shape[0]
    DC = D // P
    f = np.float32
    bf = ml_dtypes.bfloat16

    def rep(v):
        return np.broadcast_to(v.astype(f), (P, D))

    def plat(v):
        return v.astype(f).reshape(DC, P).T

    cpack = np.concatenate(
        [plat(bq), plat(bk), rep(bv), rep(bo), rep(ln_g), rep(ln_b)], axis=1)
    return {
        "qT": np.ascontiguousarray(query_slice.T.astype(f).astype(bf)),
        "kT": np.ascontiguousarray(key_b.T.astype(f).astype(bf)),
        "vT": np.ascontiguousarray(value_b.T.astype(f).astype(bf)),
        "resid": np.ascontiguousarray(query_slice.astype(f)),
        "wq": np.ascontiguousarray(wq.astype(f).astype(bf)),
        "wk": np.ascontiguousarray(wk.astype(f).astype(bf)),
        "wv": np.ascontiguousarray(wv.astype(f).astype(bf)),
        "wo": np.ascontiguousarray(wo.astype(f).astype(bf)),
        "cpack": np.ascontiguousarray(cpack),
    }


_NC_CACHE = {}


def _get_nc():
    if "nc" not in _NC_CACHE:
        _NC_CACHE["nc"] = build(SQ=1024, SK=2048, D=512, H=8, num_devices=8)
    return _NC_CACHE["nc"]


def kernel(query, key, value, wq, bq, wk, bk, wv, bv, wo, bo, ln_g, ln_b):
    from concourse.bass_utils import run_bass_kernel_spmd
    query = np.asarray(query, dtype=np.float32)
    key = np.asarray(key, dtype=np.float32)
    value = np.asarray(value, dtype=np.float32)
    B, SQ_FULL, D = query.shape
    SQH = SQ_FULL // 2
    nc = _get_nc()
    in_maps = []
    for c in range(8):
        b, qh = c // 2, c % 2
        in_maps.append(make_in_map(
            query[b, qh * SQH:(qh + 1) * SQH, :], key[b], value[b],
            np.asarray(wq), np.asarray(bq), np.asarray(wk), np.asarray(bk),
            np.asarray(wv), np.asarray(bv), np.asarray(wo), np.asarray(bo),
            np.asarray(ln_g), np.asarray(ln_b)))
    res = run_bass_kernel_spmd(nc, in_maps, core_ids=list(range(8)))
    out = np.empty((B, SQ_FULL, D), np.float32)
    for c, r in enumerate(res.results):
        out[c // 2, (c % 2) * SQH:((c % 2) + 1) * SQH, :] = r["out"]
    return out


# revision 23
# speedup vs baseline: 1.0004x; 1.0004x over previous
"""Trainium2 Bass kernel for nn_CrossModalAttention (B=4, Sq=Sk=2048, D=512, H=8).

Self-contained 8-core SPMD program: core c handles batch c//2, query-half c%2
(SQ=1024 of 2048 queries). Cores fully independent, no collectives.

Schedule (v2): the ACT engine's exp over the score matrix (H*SQ*SK elements,
128 instrs of [128,1024]) is the hard floor (~133us busy). Everything else is
arranged so ACT runs saturated from ~5us on:
  - K/V/Q projections are split into fine-grained PSUM chains (K per
    (dc,512-key window), V per (128-key chunk, 4-head group), Q per (dc,
    512-query half)) emitted in deadline order BEFORE the attention loop;
    the Tile list-scheduler trickles them into PE slack between St/PV work.
  - PSUM: St double-buffered (4 banks) + per-head context accumulator
    (1 bank, 8 qs x 64 dk) + row-sum L accumulator (1 bank, via [128,1]
    matmuls against a ones column) + 2 projection-chain banks = 8.
  - C stored bf16 so the PE transposes run 1 cycle/row; transposes for head
    pair p woven in after head 2p+1 completes (reusing projection banks).
  - DVE keeps PSUM-reading work (bias adds, normalize, LN stats); the idle
    GPSIMD engine precomputes resid+bo and does half the LN finals.
  - LayerNorm final scale fused to 2 scalar_tensor_tensor ops per row-tile.
  - Input DMAs issued in parallel across SP/ACT/DVE/POOL queues.
"""
import sys
sys.path.insert(0, "/opt/trn_rl_repo")
import numpy as np

from contextlib import ExitStack

import concourse.bass as bass
import concourse.mybir as mybir
import concourse.tile as tile
from concourse import bacc
from concourse.masks import make_identity

FP32 = mybir.dt.float32
BF16 = mybir.dt.bfloat16
P = 128


def build(SQ=1024, SK=2048, D=512, H=8, num_devices=8, dbg=False):
    DK = D // H                   # 64
    DC = D // P                   # 4
    KC = SK // P                  # 16 key chunks
    NQT = SQ // P                 # 8 query subtiles
    QF = 512
    NQF = SQ // QF                # 2
    NW = SK // 512                # 4 key windows
    HG = 2                        # head groups (4 heads each) for V chains
    HPG = H // HG                 # 4
    Alu = mybir.AluOpType
    Act = mybir.ActivationFunctionType

    nc = bacc.Bacc("TRN2", target_bir_lowering=False, debug=False,
                   num_devices=num_devices)

    def din(name, shape, dt=FP32):
        return nc.dram_tensor(name, list(shape), dt, kind="ExternalInput").ap()

    qT = din("qT", (D, SQ), BF16)
    kT = din("kT", (D, SK), BF16)
    vT = din("vT", (D, SK), BF16)
    resid = din("resid", (SQ, D))
    w_dram = {n: din(n, (D, D), BF16) for n in ("wq", "wk", "wv", "wo")}
    # packed consts: [bq 0:4][bk 4:8][bv 8:520][bo 520:1032][g 1032:1544][b 1544:2056]
    cpack = din("cpack", (P, 2 * DC + 4 * D))
    out = nc.dram_tensor("out", [SQ, D], FP32, kind="ExternalOutput").ap()

    with tile.TileContext(nc) as tc, ExitStack() as ctx:
        consts = ctx.enter_context(tc.tile_pool(name="consts", bufs=1))
        acts = ctx.enter_context(tc.tile_pool(name="acts", bufs=1))
        pj_ps = ctx.enter_context(tc.tile_pool(name="pj_ps", bufs=2, space="PSUM"))
        st_ps = ctx.enter_context(tc.tile_pool(name="st_ps", bufs=2, space="PSUM"))
        c_ps = ctx.enter_context(tc.tile_pool(name="c_ps", bufs=1, space="PSUM"))
        pt_pool = ctx.enter_context(tc.tile_pool(name="pt", bufs=8))
        lr_pool = ctx.enter_context(tc.tile_pool(name="lr", bufs=3))
        ep = ctx.enter_context(tc.tile_pool(name="ep", bufs=3))
        ot_pool = ctx.enter_context(tc.tile_pool(name="ot", bufs=2))

        # ---------------- input DMAs, spread across engine queues ----------------
        qT_sb = acts.tile([P, DC, SQ], BF16, tag="qTin")
        qTr = qT.rearrange("(c p) q -> p c q", p=P)
        nc.sync.dma_start(qT_sb[:, :, 0:QF], qTr[:, :, 0:QF])
        nc.scalar.dma_start(qT_sb[:, :, QF:SQ], qTr[:, :, QF:SQ])

        wsb = {}
        for n in ("wq", "wk", "wv", "wo"):
            wsb[n] = acts.tile([P, DC, D], BF16, tag=n, name=n)
        nc.sync.dma_start(wsb["wq"][:], w_dram["wq"].rearrange("(c p) o -> p c o", p=P))

        kin = acts.tile([P, DC, SK], BF16, tag="kin")
        kTr = kT.rearrange("(c p) s -> p c s", p=P)
        nc.scalar.dma_start(kin[:, :, 0:512], kTr[:, :, 0:512])
        nc.scalar.dma_start(wsb["wk"][:], w_dram["wk"].rearrange("(c p) o -> p c o", p=P))

        vin = acts.tile([P, DC, SK], BF16, tag="vin")
        vTr = vT.rearrange("(c p) s -> p c s", p=P)
        nc.gpsimd.dma_start(vin[:, :, 0:512], vTr[:, :, 0:512])
        nc.sync.dma_start(wsb["wv"][:], w_dram["wv"].rearrange("(c p) o -> p c o", p=P))

        # consts: only bq/bk/bv needed during projections; bo/g/b loaded late
        cp_sb = consts.tile([P, 2 * DC + 4 * D], FP32, tag="cpack")
        nc.scalar.dma_start(cp_sb[:, 0:2 * DC + D], cpack[:, 0:2 * DC + D])

        bq_sb = cp_sb[:, 0:DC]
        bk_sb = cp_sb[:, DC:2 * DC]
        bv_sb = cp_sb[:, 2 * DC:2 * DC + D]
        bo_sb = cp_sb[:, 2 * DC + D:2 * DC + 2 * D]
        g_sb = cp_sb[:, 2 * DC + 2 * D:2 * DC + 3 * D]
        b_sb = cp_sb[:, 2 * DC + 3 * D:2 * DC + 4 * D]

        ident = consts.tile([P, P], BF16, tag="ident")
        make_identity(nc, ident[:])
        ones_bf = consts.tile([P, 1], BF16, tag="ones")
        nc.vector.memset(ones_bf[:], 1.0)
        eps_sb = consts.tile([P, 1], FP32, tag="eps")
        nc.vector.memset(eps_sb[:], 1e-5)

        # ---------------- resident activations ----------------
        Qt = acts.tile([P, DC, SQ], BF16, tag="Qt")
        Kt = acts.tile([P, DC, SK], BF16, tag="Kt")
        V = acts.tile([P, KC, H, DK + 1], BF16, tag="V")
        nc.vector.memset(V[:, :, :, DK], 1.0)
        C = acts.tile([P, NQT, D], BF16, tag="C")
        Ct = acts.tile([P, DC, SQ], BF16, tag="Ct")
        t0_all = acts.tile([P, NQT, D], FP32, tag="t0")
        rs2_all = acts.tile([P, NQT, D], FP32, tag="rs2")
        mv_all = acts.tile([P, NQT, 2], FP32, tag="mv")
        sdev = acts.tile([P, NQT], FP32, tag="sdev")
        rstd_all = acts.tile([P, NQT], FP32, tag="rstd")

        # ---------------- projection chains ----------------
        def q_chain(dc, qf):
            ps = pj_ps.tile([P, D], FP32, tag="pj", name=f"q{dc}_{qf}")[:, 0:QF]
            for i in range(DC):
                nc.tensor.matmul(ps, lhsT=wsb["wq"][:, i, dc * P:(dc + 1) * P],
                                 rhs=qT_sb[:, i, qf * QF:(qf + 1) * QF],
                                 start=(i == 0), stop=(i == DC - 1))
            nc.vector.tensor_scalar_add(Qt[:, dc, qf * QF:(qf + 1) * QF], ps,
                                        bq_sb[:, dc:dc + 1])

        def k_chain(dc, w):
            ps = pj_ps.tile([P, D], FP32, tag="pj", name=f"k{dc}_{w}")[:, 0:512]
            for i in range(DC):
                nc.tensor.matmul(ps, lhsT=wsb["wk"][:, i, dc * P:(dc + 1) * P],
                                 rhs=kin[:, i, w * 512:(w + 1) * 512],
                                 start=(i == 0), stop=(i == DC - 1))
            nc.vector.tensor_scalar_add(Kt[:, dc, w * 512:(w + 1) * 512], ps,
                                        bk_sb[:, dc:dc + 1])

        def v_chain(sc, hg):
            cw = hg * HPG * DK                      # 256-col offset
            ps = pj_ps.tile([P, D], FP32, tag="pj",
                            name=f"v{sc}_{hg}")[:, 0:HPG * DK]
            for i in range(DC):
                nc.tensor.matmul(ps, lhsT=vin[:, i, sc * P:(sc + 1) * P],
                                 rhs=wsb["wv"][:, i, cw:cw + HPG * DK],
                                 start=(i == 0), stop=(i == DC - 1))
            nc.vector.tensor_tensor(
                V[:, sc, hg * HPG:(hg + 1) * HPG, 0:DK],
                ps.rearrange("p (h d) -> p h d", d=DK),
                bv_sb[:, cw:cw + HPG * DK].rearrange("p (h d) -> p h d", d=DK),
                Alu.add)

        # prologue: minimum to start (h0, kc0)
        q_chain(0, 0)
        q_chain(0, 1)
        k_chain(0, 0)
        for sc in range(4):
            v_chain(sc, 0)
        # background, deadline order (scheduler trickles into PE slack);
        # later kin/vin windows DMA'd just ahead of their first consumer
        nc.scalar.dma_start(kin[:, :, 512:1024], kTr[:, :, 512:1024])
        nc.gpsimd.dma_start(vin[:, :, 512:1024], vTr[:, :, 512:1024])
        k_chain(0, 1)
        for sc in range(4, 8):
            v_chain(sc, 0)
        nc.scalar.dma_start(kin[:, :, 1024:1536], kTr[:, :, 1024:1536])
        nc.gpsimd.dma_start(vin[:, :, 1024:1536], vTr[:, :, 1024:1536])
        k_chain(0, 2)
        for sc in range(8, 12):
            v_chain(sc, 0)
        nc.scalar.dma_start(kin[:, :, 1536:2048], kTr[:, :, 1536:2048])
        nc.gpsimd.dma_start(vin[:, :, 1536:2048], vTr[:, :, 1536:2048])
        k_chain(0, 3)
        for sc in range(12, 16):
            v_chain(sc, 0)
        q_chain(1, 0)
        q_chain(1, 1)
        for w in range(NW):
            k_chain(1, w)
        for sc in range(16):
            v_chain(sc, 1)
        q_chain(2, 0)
        q_chain(2, 1)
        for w in range(NW):
            k_chain(2, w)
        q_chain(3, 0)
        q_chain(3, 1)
        for w in range(NW):
            k_chain(3, w)

        # ---------------- attention ----------------
        JB = 4                      # qs per cps bank (65 f32 each)
        NQB = NQT // JB

        def emit_transpose(dc):
            for qs in range(NQT):
                tp = pj_ps.tile([P, P], BF16, tag="pj", name=f"tp{dc}_{qs}")
                nc.tensor.transpose(tp[:], C[:, qs, dc * P:(dc + 1) * P], ident[:])
                nc.vector.tensor_copy(out=Ct[:, dc, qs * P:(qs + 1) * P], in_=tp[:])

        for h in range(H):
            dc_h = h // 2
            off = (h % 2) * DK
            cps = c_ps.tile([P, NQB, 512], FP32, tag="cps", name=f"cps{h}")
            for kc in range(KC):
                st = st_ps.tile([P, SQ], FP32, tag="st")
                for qf in range(NQF):
                    nc.tensor.matmul(
                        st[:, qf * QF:(qf + 1) * QF],
                        lhsT=Kt[off:off + DK, dc_h, kc * P:(kc + 1) * P],
                        rhs=Qt[off:off + DK, dc_h, qf * QF:(qf + 1) * QF],
                        start=True, stop=True)
                pt = pt_pool.tile([P, SQ], BF16, tag="pt")
                nc.scalar.activation(pt[:], st[:], Act.Exp, scale=0.125)
                for qs in range(NQT):
                    jcol = (qs % JB) * (DK + 1)
                    nc.tensor.matmul(cps[:, qs // JB, jcol:jcol + DK + 1],
                                     lhsT=pt[:, qs * P:(qs + 1) * P],
                                     rhs=V[:, kc, h, :],
                                     start=(kc == 0 and qs % JB == 0),
                                     stop=(kc == KC - 1 and qs % JB == JB - 1))
            cview = cps[:, :, 0:JB * (DK + 1)].rearrange(
                "p b (j x) -> p b j x", x=DK + 1)
            lr = lr_pool.tile([P, NQB, JB, 1], FP32, tag="lr")
            nc.vector.reciprocal(lr[:], cview[:, :, :, DK:DK + 1])
            nc.vector.tensor_tensor(
                C[:, :, h * DK:(h + 1) * DK].rearrange(
                    "p (b j) d -> p b j d", j=JB),
                cview[:, :, :, 0:DK],
                lr[:].to_broadcast((P, NQB, JB, DK)),
                Alu.mult)
            if h == 5:
                # prefetch epilogue inputs while attention still runs
                nc.gpsimd.dma_start(cp_sb[:, 2 * DC + D:],
                                    cpack[:, 2 * DC + D:])
                nc.gpsimd.dma_start(wsb["wo"][:],
                                    w_dram["wo"].rearrange("(c p) o -> p c o", p=P))
                for qs in range(NQT):
                    rs = ep.tile([P, D], FP32, tag="rs")
                    nc.sync.dma_start(rs[:], resid[qs * P:(qs + 1) * P, :])
                    nc.gpsimd.tensor_tensor(rs2_all[:, qs, :], rs[:], bo_sb,
                                            Alu.add)
            if h % 2 == 1:
                emit_transpose(h // 2)

        # ---------------- out-proj + layernorm ----------------
        def ln_final(qs):
            y = ep.tile([P, D], FP32, tag="y")
            nc.vector.scalar_tensor_tensor(
                y[:], t0_all[:, qs, :], mv_all[:, qs, 0:1],
                g_sb, Alu.subtract, Alu.mult)
            ot = ot_pool.tile([P, D], FP32, tag="ot")
            nc.vector.scalar_tensor_tensor(
                ot[:], y[:], rstd_all[:, qs:qs + 1],
                b_sb, Alu.mult, Alu.add)
            nc.sync.dma_start(out[qs * P:(qs + 1) * P, :], ot[:])

        HQ = NQT // 2
        for qs in range(NQT):
            ps = st_ps.tile([P, SQ], FP32, tag="st", name=f"ops{qs}")[:, 0:D]
            for dc in range(DC):
                nc.tensor.matmul(ps, lhsT=Ct[:, dc, qs * P:(qs + 1) * P],
                                 rhs=wsb["wo"][:, dc, :],
                                 start=(dc == 0), stop=(dc == DC - 1))
            t0 = t0_all[:, qs, :]
            nc.vector.tensor_tensor(t0, ps, rs2_all[:, qs, :], Alu.add)
            st6 = ep.tile([P, 6], FP32, tag="st6")
            nc.vector.bn_stats(st6[:], t0)
            nc.vector.bn_aggr(mv_all[:, qs, :], st6[:])
            if qs == HQ - 1:
                # finalize first half while PE runs the remaining chains
                nc.scalar.activation(sdev[:, 0:HQ], mv_all[:, 0:HQ, 1],
                                     Act.Sqrt, bias=eps_sb[:])
                nc.vector.reciprocal(rstd_all[:, 0:HQ], sdev[:, 0:HQ])
                for q2 in range(HQ):
                    ln_final(q2)

        nc.scalar.activation(sdev[:, HQ:NQT], mv_all[:, HQ:NQT, 1],
                             Act.Sqrt, bias=eps_sb[:])
        nc.vector.reciprocal(rstd_all[:, HQ:NQT], sdev[:, HQ:NQT])
        for qs in range(HQ, NQT):
            ln_final(qs)

        if dbg:
            for nm, t in (("dbg_C", C), ("dbg_Ct", Ct)):
                dt_ = nc.dram_tensor(nm, list(t[:].shape), BF16,
                                     kind="ExternalOutput").ap()
                nc.sync.dma_start(dt_, t[:])
            for nm, t in (("dbg_t0", t0_all), ("dbg_rs2", rs2_all),
                          ("dbg_mv", mv_all), ("dbg_rstd", rstd_all)):
                dt_ = nc.dram_tensor(nm, list(t[:].shape), FP32,
                                     kind="ExternalOutput").ap()
                nc.sync.dma_start(dt_, t[:])

    nc.compile()
    return nc


def make_in_map(query_slice, key_b, value_b, wq, bq, wk, bk, wv, bv, wo, bo,
                ln_g, ln_b):
    """Host-side shard prep for one core. query_slice [SQ, D]; key_b/value_b [SK, D]."""
    import ml_dtypes
    D = wq.shape[0]
    DC = D // P
    f = np.float32
    bf = ml_dtypes.bfloat16

    def rep(v):
        return np.broadcast_to(v.astype(f), (P, D))

    def plat(v):
        return v.astype(f).reshape(DC, P).T

    cpack = np.concatenate(
        [plat(bq), plat(bk), rep(bv), rep(bo), rep(ln_g), rep(ln_b)], axis=1)
    return {
        "qT": np.ascontiguousarray(query_slice.T.astype(f).astype(bf)),
        "kT": np.ascontiguousarray(key_b.T.astype(f).astype(bf)),
        "vT": np.ascontiguousarray(value_b.T.astype(f).astype(bf)),
        "resid": np.ascontiguousarray(query_slice.astype(f)),
        "wq": np.ascontiguousarray(wq.astype(f).astype(bf)),
        "wk": np.ascontiguousarray(wk.astype(f).astype(bf)),
        "wv": np.ascontiguousarray(wv.astype(f).astype(bf)),
        "wo": np.ascontiguousarray(wo.astype(f).astype(bf)),
        "cpack": np.ascontiguousarray(cpack),
    }


_NC_CACHE = {}


def _get_nc():
    if "nc" not in _NC_CACHE:
        _NC_CACHE["nc"] = build(SQ=1024, SK=2048, D=512, H=8, num_devices=8)
    return _NC_CACHE["nc"]


def kernel(query, key, value, wq, bq, wk, bk, wv, bv, wo, bo, ln_g, ln_b):
    from concourse.bass_utils import run_bass_kernel_spmd
    query = np.asarray(query, dtype=np.float32)
    key = np.asarray(key, dtype=np.float32)
    value = np.asarray(value, dtype=np.float32)
    B, SQ_FULL, D = query.shape
    SQH = SQ_FULL // 2
    nc = _get_nc()
    in_maps = []
    for c in range(8):
        b, qh = c // 2, c % 2
        in_maps.append(make_in_map(
            query[b, qh * SQH:(qh + 1) * SQH, :], key[b], value[b],
            np.asarray(wq), np.asarray(bq), np.asarray(wk), np.asarray(bk),
            np.asarray(wv), np.asarray(bv), np.asarray(wo), np.asarray(bo),
            np.asarray(ln_g), np.asarray(ln_b)))
    res = run_bass_kernel_spmd(nc, in_maps, core_ids=list(range(8)))
    out = np.empty((B, SQ_FULL, D), np.float32)
    for c, r in enumerate(res.results):
        out[c // 2, (c % 2) * SQH:((c % 2) + 1) * SQH, :] = r["out"]
    return out
